# revision 1
# baseline (speedup 1.0000x reference)
"""Trainium2 Bass kernel for ragged phonology-embedding mean + position add.

Reference semantics (per (b, s)):
    out[b, s, :] = mean_{g < len[b,s]} table[tok[b,s,g], :] + pos[s, :]

Strategy (data-parallel over B across 8 cores, tables replicated). Per core
there are 16 output tiles of 128 rows; each tile's masked ragged mean is a
block-sparse matmul. Two compute paths, split to balance two independent
engine bottlenecks (measured: dma_gather costs ~9-11ns/index of GpSimd/Q7
descriptor-gen time regardless of bytes moved; TensorE is otherwise idle):

  - gather path (Q7-bound): dma_gather the deduped token rows into SBUF
    [128 packed indices, D]; W[p, m] = count/len turns the ragged mean
    into PSUM-accumulated [128x128]@[128x512] matmuls. Gather tiles are
    processed in PAIRS sharing one call: tokens ordered [A-only, shared,
    B-only] so the union (~20% smaller than the two deduped sets) is
    gathered once and only boundary chunks matmul into both PSUM tiles.
  - one-hot path (PE-bound): keep the whole bf16 table resident in SBUF
    and compute the tile as C_t.T @ table with C_t [VOCAB, 128] the
    host-built count/len matrix - 16 K-tiles, no gather at all.

VectorE fuses the position add into the PSUM->SBUF copy; outputs are
written bf16 and upcast on host. PSUM accumulates f32 throughout.
"""

import os
import numpy as np
import ml_dtypes

import concourse.bass as bass
import concourse.bacc as bacc
import concourse.mybir as mybir
import concourse.tile as tile
from concourse.bass_utils import run_bass_kernel_spmd

B, S, G = 128, 128, 8
VOCAB, D = 2048, 1024
NCORES = 8
BPC = B // NCORES          # batches per core
R = BPC * S                # rows (b,s pairs) per core
P = 128
NT = R // P                # output tiles per core
KT = VOCAB // P            # K-tiles in the one-hot path
MAXCH = 8                  # dma_gather HW cap: 1024 indices per call

# engine-cost model for the path split (ns), calibrated on HW traces
Q7_START_NS = 21_000.0     # preamble + Q7 ucode load before first real call
Q7_NS_PER_IDX = 7.8        # serial descriptor-gen per gathered index (calibrated)
Q7_CALL_FIXED_NS = 700.0
Q7_TAIL_NS = 10_000.0      # last call's SDMA drain + its matmuls/DVE
PE_START_NS = 17_000.0     # first C/tablek chunk landing
PE_NS_PER_MM512 = 215.0    # warm N=512 bf16 issue cadence


def _cdiv(a, b):
    return -(-a // b)


def _prepare(phon_tokens, group_len_raw):
    toks = np.asarray(phon_tokens).astype(np.int64).reshape(B, S, G)
    lens = (np.asarray(group_len_raw).astype(np.int64) + 1).reshape(B, S)
    assert lens.min() >= 1 and lens.max() <= G
    assert toks.min() >= 0 and toks.max() < VOCAB

    toks_c = toks.reshape(NCORES, R, G)
    lens_c = lens.reshape(NCORES, R)

    # dedup tokens per (core, tile); weights carry count/len so each
    # distinct token contributes once.
    uniqs = {}
    wmats = {}
    nuniq = np.zeros((NCORES, NT), int)
    for c in range(NCORES):
        for t in range(NT):
            tl = toks_c[c, t * P:(t + 1) * P]
            ll = lens_c[c, t * P:(t + 1) * P]
            valid = np.arange(G)[None, :] < ll[:, None]
            flat = tl[valid]
            pair = np.repeat(np.arange(P), ll)
            uniq, inv = np.unique(flat, return_inverse=True)
            wm = np.zeros((uniq.size, P), np.float32)
            np.add.at(wm, (inv, pair), 1.0 / ll[pair])
            uniqs[c, t] = uniq
            wmats[c, t] = wm
            nuniq[c, t] = uniq.size

    Mt = np.maximum(_cdiv(nuniq, P).max(axis=0), 1).astype(int)  # [NT]

    # Path split: pick x one-hot tiles (PE-only) vs gather tiles (Q7-bound),
    # simulating the pair-merged gather plan, minimizing max(engine end).
    order = np.argsort(-Mt, kind="stable")

    def _pairing(gtiles):
        groups = []
        i = 0
        while i < len(gtiles):
            if i + 1 < len(gtiles):
                ta, tb = gtiles[i], gtiles[i + 1]
                nun = max(
                    np.union1d(uniqs[c, ta], uniqs[c, tb]).size
                    for c in range(NCORES)
                )
                nchp = _cdiv(nun, P)
                if nchp <= MAXCH:
                    groups.append(((ta, tb), nchp))
                    i += 2
                    continue
            groups.append(((gtiles[i],), int(Mt[gtiles[i]])))
            i += 1
        return groups

    best_x, best_cost = 0, None
    for x in range(NT + 1):
        gtiles = [t for t in range(NT) if t not in set(order[:x].tolist())]
        grps = _pairing(gtiles)
        q7 = Q7_START_NS + sum(
            nch * P * Q7_NS_PER_IDX + Q7_CALL_FIXED_NS for _, nch in grps
        ) + (Q7_TAIL_NS if grps else 0.0)
        # entries per pair call ~= nch + boundary overlap (~4); single = nch
        mm = 32 * x + sum(
            2 * (nch + (4 if len(g) == 2 else 0)) for g, nch in grps
        )
        pe = PE_START_NS + mm * PE_NS_PER_MM512
        cost = max(pe, q7)
        if best_cost is None or cost < best_cost:
            best_x, best_cost = x, cost
    kx = int(os.environ.get("KX", "-1"))
    if kx >= 0:
        best_x = kx
    onehot_tiles = sorted(order[:best_x].tolist())
    gather_tiles = [t for t in range(NT) if t not in onehot_tiles]
    nx = len(onehot_tiles)

    groups = [g for g, _ in _pairing(gather_tiles)]

    # Build call descriptors (shared program structure across cores) and the
    # per-core token lists.
    calls = []            # dicts: nch, idx_base, entries [(j, t, first, last)]
    toklists = {}         # (core, call_idx) -> np.int64 token list (padded)
    chunk_off = 0
    entry_off = 0
    for gidx, grp in enumerate(groups):
        if len(grp) == 1:
            (t,) = grp
            nch = int(Mt[t])
            entries = [(j, t) for j in range(nch)]
        else:
            ta, tb = grp
            nA = np.zeros(NCORES, int)
            nBs = np.zeros(NCORES, int)
            nU = np.zeros(NCORES, int)
            for c in range(NCORES):
                ua, ub = uniqs[c, ta], uniqs[c, tb]
                sh = np.intersect1d(ua, ub, assume_unique=True)
                aonly = np.setdiff1d(ua, sh, assume_unique=True)
                bonly = np.setdiff1d(ub, sh, assume_unique=True)
                toklists[c, gidx] = np.concatenate([aonly, sh, bonly])
                nA[c] = aonly.size + sh.size
                nBs[c] = aonly.size
                nU[c] = aonly.size + sh.size + bonly.size
            nch = int(_cdiv(nU, P).max())
            entries = []
            for j in range(nch):
                if j * P < nA.max():
                    entries.append((j, ta))
                if (j + 1) * P > nBs.min():
                    entries.append((j, tb))
        if len(grp) == 1:
            for c in range(NCORES):
                toklists[c, gidx] = uniqs[c, grp[0]]
        calls.append(dict(
            nch=nch, idx_base=chunk_off, entry_base=entry_off,
            entries=entries, grp=grp,
        ))
        chunk_off += nch
        entry_off += len(entries)
    total_chunks = chunk_off
    total_entries = entry_off

    # Split the final call so the kernel tail (data drain + matmuls of the
    # very last call) is as short as possible; its PSUM accumulation spans
    # the two calls.
    if calls and len(calls[-1]["grp"]) == 1 and calls[-1]["nch"] >= 4:
        last = calls.pop()
        (t,) = last["grp"]
        nch = last["nch"]
        k = nch - 2
        for c0, cn in [(0, k), (k, nch - k)]:
            calls.append(dict(
                nch=cn, idx_base=last["idx_base"] + c0,
                entry_base=last["entry_base"] + c0,
                entries=[(j, t) for j in range(cn)],
                grp=(t,), wm_row0=c0 * P,
            ))
        gidx0 = len(calls) - 2
        for c in range(NCORES):
            full = uniqs[c, t]
            toklists[c, gidx0] = full[:min(k * P, full.size)]
            toklists[c, gidx0 + 1] = full[min(k * P, full.size):]

    # annotate first/last per tile GLOBALLY (accumulation groups may span
    # split calls)
    gfirst = {}
    glast = {}
    for ci, call in enumerate(calls):
        for e, (j, t) in enumerate(call["entries"]):
            gfirst.setdefault(t, (ci, e))
            glast[t] = (ci, e)
    for ci, call in enumerate(calls):
        call["first"] = {
            t: e for t, (c_, e) in gfirst.items() if c_ == ci
        }
        call["last"] = {
            t: e for t, (c_, e) in glast.items() if c_ == ci
        }

    wdt = ml_dtypes.bfloat16
    idx_all = np.zeros((NCORES, max(total_chunks, 1) * P), np.int64)
    w_all = np.zeros((NCORES, max(total_entries, 1), P, P), np.float32)
    c_all = np.zeros((NCORES, max(nx, 1), VOCAB, P), np.float32)
    for c in range(NCORES):
        for xt, t in enumerate(onehot_tiles):
            uniq = uniqs[c, t]
            c_all[c, xt, uniq, :] = wmats[c, t]
        for gidx, call in enumerate(calls):
            toks_l = toklists[c, gidx]
            nv = toks_l.size
            idx_all[c, call["idx_base"] * P:call["idx_base"] * P + nv] = toks_l
            if len(call["grp"]) == 1:
                (t,) = call["grp"]
                wm = wmats[c, t]
                r0 = call.get("wm_row0", 0)
                for e, (j, tt) in enumerate(call["entries"]):
                    lo, hi = j * P, min((j + 1) * P, nv)
                    if lo < hi:
                        w_all[c, call["entry_base"] + e, :hi - lo] = (
                            wm[r0 + lo:r0 + hi]
                        )
            else:
                ta, tb = call["grp"]
                ua, ub = uniqs[c, ta], uniqs[c, tb]
                # per-core side boundaries in the ordered list
                in_a = np.isin(toks_l, ua, assume_unique=True)
                in_b = np.isin(toks_l, ub, assume_unique=True)
                for e, (j, tt) in enumerate(call["entries"]):
                    lo, hi = j * P, min((j + 1) * P, nv)
                    if lo >= hi:
                        continue
                    seg = toks_l[lo:hi]
                    side = in_a[lo:hi] if tt == ta else in_b[lo:hi]
                    if not side.any():
                        continue
                    wm = wmats[c, tt]
                    uu = ua if tt == ta else ub
                    rows = np.searchsorted(uu, seg[side])
                    w_all[c, call["entry_base"] + e, np.nonzero(side)[0]] = (
                        wm[rows]
                    )

    idx_maps, w_maps, c_maps = [], [], []
    for c in range(NCORES):
        idxw = np.tile(idx_all[c].reshape(-1, 16).T, (8, 1)).astype(np.int16)
        idx_maps.append(np.ascontiguousarray(idxw))
        wf = w_all[c].transpose(1, 0, 2).reshape(P, -1).astype(wdt)
        w_maps.append(np.ascontiguousarray(wf))
        cf = (
            c_all[c]
            .reshape(max(nx, 1), KT, P, P)
            .transpose(2, 0, 1, 3)
            .reshape(P, -1)
            .astype(wdt)
        )
        c_maps.append(np.ascontiguousarray(cf))

    meta = dict(
        onehot_tiles=onehot_tiles, calls=calls,
        total_chunks=total_chunks, total_entries=total_entries,
    )
    return meta, idx_maps, w_maps, c_maps


def _build_nc(meta):
    mdt = mybir.dt.bfloat16
    f32 = mybir.dt.float32
    onehot_tiles = meta["onehot_tiles"]
    calls = meta["calls"]
    total_chunks = max(meta["total_chunks"], 1)
    total_entries = max(meta["total_entries"], 1)
    nx = len(onehot_tiles)
    max_entries = max((len(c["entries"]) for c in calls), default=1)

    nc = bacc.Bacc("TRN2", target_bir_lowering=False, debug=False)

    table_d = nc.dram_tensor("table", [VOCAB, D], mdt, kind="ExternalInput")
    tablek_d = nc.dram_tensor("tablek", [P, KT * D], mdt, kind="ExternalInput")
    pos_d = nc.dram_tensor("pos", [P, D], f32, kind="ExternalInput")
    idx_d = nc.dram_tensor("idxs", [P, total_chunks * 8], mybir.dt.int16,
                           kind="ExternalInput")
    w_d = nc.dram_tensor("wmat", [P, total_entries * P], mdt,
                         kind="ExternalInput")
    c_d = nc.dram_tensor("cmat", [P, max(nx, 1) * KT * P], mdt,
                         kind="ExternalInput")
    out_d = nc.dram_tensor("out", [R, D], mdt, kind="ExternalOutput")

    # Order tiles by expected readiness: the PE stream is in-order, so a
    # gather call's matmuls must not be emitted before its data can land
    # (Q7 ucode load ~16us + serial desc-gen + SDMA drain) or PE stalls.
    # Front-load one-hot tiles (ready as soon as their C lands) and release
    # each gather call when the PE cursor passes its data-ready estimate.
    q7_t = 21_000.0
    ready = []
    for call in calls:
        q7_t += call["nch"] * P * 8.75 + 700.0
        ready.append(q7_t + 6_000.0)
    pe_t = 17_000.0
    sched = []
    gi, oi = list(range(len(calls))), list(range(nx))
    while gi or oi:
        if gi and (not oi or ready[gi[0]] <= pe_t):
            c = gi.pop(0)
            sched.append(("g", c))
            pe_t = max(pe_t, ready[c]) + len(calls[c]["entries"]) * 2 * 215.0
        else:
            sched.append(("o", oi.pop(0)))
            pe_t += KT * 2 * 215.0

    with tile.TileContext(nc) as tc:
        with (
            tc.tile_pool(name="const", bufs=1) as cpool,
            tc.tile_pool(name="gather", bufs=6) as gpool,
            tc.tile_pool(name="wpool", bufs=4) as wpool,
            tc.tile_pool(name="cpool2", bufs=3) as cwpool,
            tc.tile_pool(name="osb", bufs=6) as opool,
            tc.tile_pool(name="psum", bufs=4, space=bass.MemorySpace.PSUM) as ppool,
        ):
            warm_idx = cpool.tile([P, 8], mybir.dt.int16)
            nc.gpsimd.memset(warm_idx[:], 0)
            warm_gt = cpool.tile([P, 1, 128], mdt)
            nc.gpsimd.dma_gather(
                warm_gt[:, :, :], table_d[:, :128], warm_idx[:],
                num_idxs=P, num_idxs_reg=P, elem_size=128, elem_step=D,
            )
            idx_sb = cpool.tile([P, total_chunks * 8], mybir.dt.int16)
            nc.sync.dma_start(idx_sb[:], idx_d[:])
            ct0 = None
            if nx:
                ct0 = cpool.tile([P, KT * P], mdt)
                nc.sync.dma_start(ct0[:], c_d[:, :KT * P])
            tk_sb = cpool.tile([P, KT * D], mdt)
            if nx:
                kstep = 4 * D
                for kt in range(0, KT * D, kstep):
                    nc.sync.dma_start(
                        tk_sb[:, kt:kt + kstep],
                        tablek_d[:, kt:kt + kstep],
                    )
            pos_sb = cpool.tile([P, D], f32)
            nc.sync.dma_start(pos_sb[:], pos_d[:])

            nregs = {}
            psums = {}
            for kind, item in sched:
                if kind == "g":
                    call = calls[item]
                    nch = call["nch"]
                    b0 = call["idx_base"]
                    n_idx = nch * P
                    if n_idx not in nregs:
                        nregs[n_idx] = nc.gpsimd.to_reg(n_idx)
                    gt = gpool.tile([P, MAXCH, D], mdt, tag="gt")
                    nc.gpsimd.dma_gather(
                        gt[:, :nch, :],
                        table_d[:],
                        idx_sb[:, b0 * 8:(b0 + nch) * 8],
                        num_idxs=n_idx,
                        num_idxs_reg=nregs[n_idx],
                        elem_size=D,
                    )
                    ne = len(call["entries"])
                    wt = wpool.tile([P, max_entries * P], mdt, tag="wt")
                    eb = call["entry_base"]
                    nc.sync.dma_start(
                        wt[:, :ne * P], w_d[:, eb * P:(eb + ne) * P]
                    )
                    for e, (j, t) in enumerate(call["entries"]):
                        if call["first"].get(t, -1) == e:
                            psums[t] = ppool.tile([P, D], f32, tag="ps", name="ps")
                        for h in range(0, D, 512):
                            nc.tensor.matmul(
                                psums[t][:, h:h + 512],
                                lhsT=wt[:, e * P:(e + 1) * P],
                                rhs=gt[:, j, h:h + 512],
                                start=(call["first"].get(t, -1) == e),
                                stop=(call["last"].get(t, -1) == e),
                            )
                        if call["last"].get(t, -1) == e:
                            ot = opool.tile([P, D], mdt, tag="ot")
                            nc.vector.tensor_tensor(
                                ot[:], psums[t][:], pos_sb[:],
                                op=mybir.AluOpType.add,
                            )
                            nc.sync.dma_start(
                                out_d[t * P:(t + 1) * P, :], ot[:]
                            )
                else:
                    xt = item
                    t = onehot_tiles[xt]
                    if xt == 0 and ct0 is not None:
                        ct = ct0
                    else:
                        ct = cwpool.tile([P, KT * P], mdt, tag="ct")
                        nc.sync.dma_start(
                            ct[:], c_d[:, xt * KT * P:(xt + 1) * KT * P]
                        )
                    psum_t = ppool.tile([P, D], f32, tag="ps")
                    for k in range(KT):
                        for h in range(0, D, 512):
                            nc.tensor.matmul(
                                psum_t[:, h:h + 512],
                                lhsT=ct[:, k * P:(k + 1) * P],
                                rhs=tk_sb[:, k * D + h:k * D + h + 512],
                                start=(k == 0),
                                stop=(k == KT - 1),
                            )
                    ot = opool.tile([P, D], mdt, tag="ot")
                    nc.vector.tensor_tensor(
                        ot[:], psum_t[:], pos_sb[:], op=mybir.AluOpType.add
                    )
                    nc.sync.dma_start(out_d[t * P:(t + 1) * P, :], ot[:])
    nc.compile()
    return nc


def run(inputs, trace=False, tmpdir=None):
    """Returns (out [B,S,D] f32, BassKernelResults)."""
    meta, idx_maps, w_maps, c_maps = _prepare(
        inputs["phon_tokens"], inputs["group_len_raw"]
    )
    wdt = ml_dtypes.bfloat16
    table_np = np.ascontiguousarray(
        np.asarray(inputs["phon_emb_table"]).astype(wdt)
    )
    tablek_np = np.ascontiguousarray(
        table_np.reshape(KT, P, D).transpose(1, 0, 2).reshape(P, KT * D)
    )
    pos_np = np.ascontiguousarray(
        np.asarray(inputs["pos_emb_table"]).astype(np.float32)
    )

    nc = _build_nc(meta)
    in_maps = [
        {
            "table": table_np, "tablek": tablek_np, "pos": pos_np,
            "idxs": idx_maps[c], "wmat": w_maps[c], "cmat": c_maps[c],
        }
        for c in range(NCORES)
    ]
    res = run_bass_kernel_spmd(
        nc, in_maps, core_ids=list(range(NCORES)), trace=trace, tmpdir=tmpdir
    )
    out = np.empty((B, S, D), np.float32)
    for c in range(NCORES):
        out[c * BPC:(c + 1) * BPC] = (
            res.results[c]["out"].astype(np.float32).reshape(BPC, S, D)
        )
    return out, res


def kernel(**inputs) -> np.ndarray:
    out, _ = run(inputs, trace=False)
    return out

